# revision 1
# baseline (speedup 1.0000x reference)
"""CapsuleModel2 segment-reduce kernel for 8 TRN2 NeuronCores.

Math (per reference.py):
    feats  = class_capsules.reshape(65536, 272)[point_idx]        # [P, 272]
    sums   = segment_sum(feats, segment_ids, 4096)                # [4096, 272]
    counts = segment_sum(ones)                                    # [4096]
    out    = sigmoid((sums / max(counts,1)) @ W + b)              # [4096, 19]

Key identity used: (sums @ W) = segment_sum(feats @ W), so we project the
65536x272 grid down to 19 channels FIRST (on the PE), then gather 20-wide
rows (19 channels + a constant-1 count column) instead of 272-wide rows.

Distribution (table-sharded):
  - core k owns grid rows [k*8192, (k+1)*8192) and the points that hit them
  - each core computes partial sums over ALL 4096 segments via a one-hot
    matmul (exploits segment-sortedness: 128 consecutive points of a core
    stay inside one 128-segment block after per-block padding)
  - ReduceScatter(add) gives core k the totals for segments [k*512,(k+1)*512)
  - finalize: divide by counts, add bias, sigmoid

Pipeline per core:
  A) proj:   table[8192, 128]bf16 = gridT.T @ [W | 1] (cols 20..127 zero)
  B) gather: dma_gather point rows (padded per 128-seg block, idx 0 = pad)
     reduce: psum[128seg, 20] += onehot(segrel)^T @ X  (18 chunks per block)
  C) ReduceScatter + finalize + write [512, 19]
"""

import sys

for _p in ('/opt/trn_rl_repo',):
    if _p not in sys.path:
        sys.path.insert(0, _p)

import numpy as np
import ml_dtypes

import concourse.bacc as bacc
import concourse.bass as bass
import concourse.mybir as mybir
import concourse.tile as tile

BF16 = mybir.dt.bfloat16
F32 = mybir.dt.float32
I16 = mybir.dt.int16
F16 = mybir.dt.float16

NCORE = 8
GRID = 65536
GPC = GRID // NCORE          # 8192 grid rows per core
D = 272                      # capsule feature dim
NCH = 19                     # output channels
NW = NCH + 1                 # + count column
NSEG = 4096
SEGBLK = 128                 # segments per block
NBLK = NSEG // SEGBLK        # 32
CAP = 2304                   # padded points per (core, block); actual max 2181
CPB = CAP // 128             # 18 chunks per block
NCHUNK = NBLK * CPB          # 576 chunks per core
NIDX = NCHUNK * 128          # 73728 gather slots per core
NSLICE = 24
CHUNKS_PER_SLICE = NCHUNK // NSLICE   # 72
IDX_PER_SLICE = NIDX // NSLICE        # 9216
BLKS_PER_SLICE = NBLK // NSLICE       # 4
ELEM = 128                   # table row width (bf16) = 256B
SEG_PER_CORE = NSEG // NCORE  # 512
MTILE = 512                  # grid columns loaded per projection step


def build_nc(skip_collective=False, rs_inloop=False):
    nc = bacc.Bacc("TRN2", num_devices=NCORE, num_swdge_queues=4)

    gridT = nc.dram_tensor("gridT", [D, GPC], BF16, kind="ExternalInput")
    w_pack = nc.dram_tensor("w_pack", [128, 60], BF16, kind="ExternalInput")
    idx_in = nc.dram_tensor("idx", [NSLICE, 128, IDX_PER_SLICE // 16], I16,
                            kind="ExternalInput")
    segrel_in = nc.dram_tensor("segrel", [128, NCHUNK], BF16, kind="ExternalInput")
    iota_in = nc.dram_tensor("iota", [128, SEGBLK], BF16, kind="ExternalInput")
    bias_in = nc.dram_tensor("bias", [128, NCH], F32, kind="ExternalInput")
    if skip_collective:
        out_t = nc.dram_tensor("out", [NBLK, 128, NW], F16, kind="ExternalOutput")
    else:
        out_t = nc.dram_tensor("out", [SEG_PER_CORE, NCH], F32, kind="ExternalOutput")

    table = nc.dram_tensor("table", [GPC, ELEM], BF16)
    partial_d = nc.dram_tensor("partial", [NBLK, 128, NW], F16)
    rs_out = nc.dram_tensor("rs_out", [8, 64, NW], F16)

    with tile.TileContext(nc) as tc:
        with (
            tc.tile_pool(name="const", bufs=1) as cpool,
            tc.tile_pool(name="grid", bufs=3) as gpool,
            tc.tile_pool(name="ptab", bufs=2, space="PSUM") as pt_pool,
            tc.tile_pool(name="tab", bufs=4) as tpool,
            tc.tile_pool(name="idxp", bufs=1) as ipool,
            tc.tile_pool(name="dst", bufs=6) as dpool,
            tc.tile_pool(name="oh", bufs=6) as opool,
            tc.tile_pool(name="pblk", bufs=6, space="PSUM") as pb_pool,
            tc.tile_pool(name="acc", bufs=1) as apool,
            tc.tile_pool(name="fin", bufs=2) as fpool,
        ):
            # constants
            w_sb = cpool.tile([128, 60], BF16)
            nc.sync.dma_start(w_sb[:], w_pack[:])
            segrel_sb = cpool.tile([128, NCHUNK], BF16)
            nc.sync.dma_start(segrel_sb[:], segrel_in[:])
            iota_sb = cpool.tile([128, SEGBLK], BF16)
            nc.sync.dma_start(iota_sb[:], iota_in[:])
            bias_sb = cpool.tile([128, NCH], F32)
            nc.sync.dma_start(bias_sb[:], bias_in[:])

            # ---- Phase A: projection -> table ----
            ksizes = [(0, 128), (128, 128), (256, 16)]
            for mt in range(GPC // MTILE):
                gt = gpool.tile([128, 3, MTILE], BF16, tag="gt")
                for t, (k0, kn) in enumerate(ksizes):
                    nc.sync.dma_start(
                        gt[:kn, t, :],
                        gridT[k0:k0 + kn, mt * MTILE:(mt + 1) * MTILE])
                tab = tpool.tile([128, MTILE // 128, ELEM], BF16, tag="tab")
                for ms in range(MTILE // 128):
                    psum = pt_pool.tile([128, NW], F32, tag="ptab")
                    for t, (k0, kn) in enumerate(ksizes):
                        nc.tensor.matmul(
                            out=psum[:],
                            lhsT=gt[:kn, t, ms * 128:(ms + 1) * 128],
                            rhs=w_sb[:kn, t * NW:(t + 1) * NW],
                            start=(t == 0), stop=(t == 2))
                    nc.vector.memset(tab[:, ms, NCH:], 0.0)
                    nc.vector.memset(tab[:, ms, NCH:NW], 1.0)
                    nc.vector.tensor_copy(tab[:, ms, :NCH], psum[:, :NCH])
                nc.scalar.dma_start(
                    table[mt * MTILE:(mt + 1) * MTILE]
                        .rearrange("(ms p) e -> p ms e", p=128),
                    tab[:])

            # ---- Phase B: gather + one-hot reduce ----
            part_sb = apool.tile([128, NBLK * NW], F16)
            idx_all = ipool.tile([128, NSLICE, IDX_PER_SLICE // 16], I16)
            psum_b = None
            nc.sync.dma_start(idx_all[:], idx_in[:].rearrange("s p c -> p s c"))
            for s in range(NSLICE):
                idx_sb = idx_all[:, s, :]
                dst = dpool.tile([128, CHUNKS_PER_SLICE, ELEM], BF16, tag="dst")
                nc.gpsimd.dma_gather(
                    dst[:], table[:], idx_sb[:],
                    IDX_PER_SLICE, IDX_PER_SLICE, ELEM, single_packet=False,
                    queue_num=s % 4)
                oh = opool.tile([128, CHUNKS_PER_SLICE, SEGBLK], BF16, tag="oh")
                nc.vector.tensor_tensor(
                    out=oh[:],
                    in0=segrel_sb[:, s * CHUNKS_PER_SLICE:(s + 1) * CHUNKS_PER_SLICE]
                        .rearrange("p (c o) -> p c o", o=1).broadcast_to(
                            [128, CHUNKS_PER_SLICE, SEGBLK]),
                    in1=iota_sb[:].rearrange("p (o j) -> p o j", o=1).broadcast_to(
                            [128, CHUNKS_PER_SLICE, SEGBLK]),
                    op=mybir.AluOpType.is_equal)
                for g in range(CHUNKS_PER_SLICE):
                    gg = s * CHUNKS_PER_SLICE + g
                    blk, j = divmod(gg, CPB)
                    if j == 0:
                        psum_b = pb_pool.tile([128, NW], F32, tag="pblk")
                    nc.tensor.matmul(
                        out=psum_b[:],
                        lhsT=oh[:, g, :],
                        rhs=dst[:, g, :NW],
                        start=(j == 0), stop=(j == CPB - 1))
                    if j == CPB - 1:
                        nc.scalar.copy(
                            part_sb[:, blk * NW:(blk + 1) * NW], psum_b[:])
                # each 3 slices complete one 4-block RS chunk; issue the
                # store right away, but stagger the collective trigger two
                # gather-gens later so the Pool engine never stalls on it
                if s % 3 == 2:
                    h = s // 3
                    BPR = NBLK // 8
                    nc.sync.dma_start(
                        (out_t if skip_collective else partial_d)
                            [h * BPR:(h + 1) * BPR].rearrange("b p c -> p b c"),
                        part_sb[:, h * BPR * NW:(h + 1) * BPR * NW]
                            .rearrange("p (b c) -> p b c", b=BPR))
                if not skip_collective and rs_inloop and s >= 4 and (s - 4) % 3 == 0:
                    h = (s - 4) // 3
                    BPR = NBLK // 8
                    nc.gpsimd.collective_compute(
                        "ReduceScatter",
                        mybir.AluOpType.add,
                        replica_groups=[list(range(NCORE))],
                        ins=[partial_d[h * BPR:(h + 1) * BPR]],
                        outs=[rs_out[h]],
                    )

            if not skip_collective:
                BPR = NBLK // 8
                for h in ((7,) if rs_inloop else range(8)):
                    nc.gpsimd.collective_compute(
                        "ReduceScatter",
                        mybir.AluOpType.add,
                        replica_groups=[list(range(NCORE))],
                        ins=[partial_d[h * BPR:(h + 1) * BPR]],
                        outs=[rs_out[h]],
                    )

            # ---- Phase C: finalize ----
            if skip_collective:
                nc.compile_marker = None
            if not skip_collective:
              rs_flat = rs_out[:].rearrange("h p c -> (h p) c")
              for t in range(NBLK // NCORE):
                  fin16 = fpool.tile([128, NW], F16, tag="fin16")
                  nc.sync.dma_start(fin16[:], rs_flat[t * 128:(t + 1) * 128])
                  fin = fpool.tile([128, NW], F32, tag="fin")
                  nc.vector.tensor_copy(fin[:], fin16[:])
                  cnt = fpool.tile([128, 1], F32, tag="cnt")
                  nc.vector.tensor_scalar_max(cnt[:], fin[:, NCH:NW], 1.0)
                  rec = fpool.tile([128, 1], F32, tag="rec")
                  nc.vector.reciprocal(rec[:], cnt[:])
                  sc = fpool.tile([128, NCH], F32, tag="sc")
                  nc.vector.tensor_scalar_mul(sc[:], fin[:, :NCH], rec[:])
                  sc2 = fpool.tile([128, NCH], F32, tag="sc2")
                  nc.vector.tensor_add(sc2[:], sc[:], bias_sb[:])
                  og = fpool.tile([128, NCH], F32, tag="og")
                  nc.scalar.activation(og[:], sc2[:],
                                       mybir.ActivationFunctionType.Sigmoid)
                  nc.sync.dma_start(out_t[t * 128:(t + 1) * 128, :], og[:])

    nc.compile()
    return nc


def prep_inputs(class_capsules, W, b, point_idx, segment_ids, num_segments=NSEG):
    """Host-side sharding: returns in_maps (list of 8 dicts)."""
    assert int(num_segments) == NSEG
    grid = np.ascontiguousarray(class_capsules.reshape(GRID, D), np.float32)
    point_idx = np.asarray(point_idx, np.int64)
    segment_ids = np.asarray(segment_ids, np.int64)
    W = np.asarray(W, np.float32)
    b = np.asarray(b, np.float32)

    w_pack = np.zeros((128, 60), ml_dtypes.bfloat16)
    w20 = np.concatenate([W, np.zeros((D, 1), np.float32)], 1)  # [272, 19+pad]
    w_pack[:, 0:20] = w20[0:128].astype(ml_dtypes.bfloat16)
    w_pack[:, 20:40] = w20[128:256].astype(ml_dtypes.bfloat16)
    w_pack[0:16, 40:60] = w20[256:272].astype(ml_dtypes.bfloat16)

    iota = np.tile(np.arange(SEGBLK, dtype=np.float32), (128, 1)).astype(
        ml_dtypes.bfloat16)
    bias_rep = np.tile(b[None, :], (128, 1)).astype(np.float32)

    in_maps = []
    for k in range(NCORE):
        sel = (point_idx >= k * GPC) & (point_idx < (k + 1) * GPC)
        lidx = (point_idx[sel] - k * GPC).astype(np.int16)
        lseg = segment_ids[sel]          # still sorted ascending
        blk = (lseg >> 7).astype(np.int64)
        srel = (lseg & 127).astype(np.float32)
        counts = np.bincount(blk, minlength=NBLK)
        assert counts.max() <= CAP, f"core {k}: block count {counts.max()} > CAP"
        start = np.zeros(NBLK, np.int64)
        start[1:] = np.cumsum(counts)[:-1]
        rank = np.arange(lidx.size) - start[blk]
        pos = blk * CAP + rank

        idx_pad = np.zeros(NIDX, np.int16)
        srel_pad = np.full(NIDX, -1.0, np.float32)
        idx_pad[pos] = lidx
        srel_pad[pos] = srel

        segrel_arr = srel_pad.reshape(NCHUNK, 128).T.astype(ml_dtypes.bfloat16)
        idxw = np.empty((NSLICE, 128, IDX_PER_SLICE // 16), np.int16)
        for s in range(NSLICE):
            chunk = idx_pad[s * IDX_PER_SLICE:(s + 1) * IDX_PER_SLICE]
            idxw[s] = np.tile(chunk.reshape(-1, 16).T, (8, 1))

        gridT_k = np.ascontiguousarray(
            grid[k * GPC:(k + 1) * GPC].T).astype(ml_dtypes.bfloat16)

        in_maps.append({
            "gridT": gridT_k,
            "w_pack": w_pack,
            "idx": idxw,
            "segrel": np.ascontiguousarray(segrel_arr),
            "iota": iota,
            "bias": bias_rep,
        })
    return in_maps


def assemble(results):
    out = np.empty((NSEG, NCH), np.float32)
    for k in range(NCORE):
        r = results[k]["out"]  # [512, 19]: row i -> chunk h=i//64, j=i%64
        for h in range(8):
            out[4 * h * 128 + k * 64: 4 * h * 128 + (k + 1) * 64] =                 r[h * 64:(h + 1) * 64]
    return out


_NC_CACHE = {}


def kernel(class_capsules, W, b, point_idx, segment_ids, num_segments):
    """Full-input entry point: shard across 8 NeuronCores, run, reassemble."""
    from concourse.bass_utils import run_bass_kernel_spmd

    in_maps = prep_inputs(np.asarray(class_capsules), np.asarray(W),
                          np.asarray(b), np.asarray(point_idx),
                          np.asarray(segment_ids), int(num_segments))
    if "nc" not in _NC_CACHE:
        _NC_CACHE["nc"] = build_nc()
    res = run_bass_kernel_spmd(_NC_CACHE["nc"], in_maps, list(range(NCORE)))
    return assemble(res.results)



# revision 6
# speedup vs baseline: 1.0841x; 1.0841x over previous
"""CapsuleModel2 segment-reduce kernel for 8 TRN2 NeuronCores (v2).

Math (per reference):
    feats  = class_capsules.reshape(65536, 272)[point_idx]        # [P, 272]
    sums   = segment_sum(feats, segment_ids, 4096)                # [4096, 272]
    counts = segment_sum(ones)                                    # [4096]
    out    = sigmoid((sums / max(counts,1)) @ W + b)              # [4096, 19]

Identity used: (sums @ W) = segment_sum(feats @ W): project the grid to 19
channels first on the PE, then gather narrow rows.

Distribution (table-sharded):
  - core k owns grid rows [k*8192, (k+1)*8192) and the points hitting them
  - per 128-segment block, the core's points are gathered and reduced into
    psum[128seg, 19] via one-hot matmuls; partials for all 4096 segments are
    ReduceScattered so core k finalizes segments it owns
  - counts come from a host bincount (pure index arithmetic), shipped as
    reciprocals, so no count column is needed on device

Gather bandwidth tricks (the DMA queues cost ~the same per packet for 256B
and 512B):
  - each gather index fetches TWO adjacent 256B table rows (elem_step=128,
    elem_size=256 -> one 512B packet covering rows {g, g+1}); points sorted
    by grid row within a block are greedily paired so one packet often
    serves two points (slot A = row g, slot B = row g+1); unused slots are
    killed by a -1 in the one-hot segment map
  - per-block trailing pad indices are -1 and num_idxs_reg carries the true
    packet count, so padding costs no DMA traffic
"""

import sys

for _p in ('/opt/trn_rl_repo',):
    if _p not in sys.path:
        sys.path.insert(0, _p)

import numpy as np
import ml_dtypes

import bass_rust
import concourse.bacc as bacc
import concourse.bass as bass
import concourse.mybir as mybir
import concourse.tile as tile

BF16 = mybir.dt.bfloat16
F32 = mybir.dt.float32
F16 = mybir.dt.float16
I16 = mybir.dt.int16
I32 = mybir.dt.int32

NCORE = 8
GRID = 65536
GPC = GRID // NCORE          # 8192 grid rows per core
D = 272                      # capsule feature dim
NCH = 19                     # output channels
NSEG = 4096
SEGBLK = 128                 # segments per block
NBLK = NSEG // SEGBLK        # 32
CAPP = 1920                  # padded packets per (core, block); actual max 1819
CPB = CAPP // 128            # 15 packet-chunks per block
NROWS_PAD = GPC + 128        # table rows incl. zero pad row for pair overread
SEG_PER_CORE = NSEG // NCORE  # 512
MTILE = 1024                 # grid columns per projection step
import os
TRIM = os.environ.get("KTRIM", "1") == "1"   # num_idxs_reg trims pad packets
RS_INLOOP = os.environ.get("KRSIL", "1") == "1"  # stagger RS inside phase B


def _pair_ap(t):
    """Overlapping AP over table [NROWS_PAD,128]: row stride 128 elems,
    row view 256 elems -> one 512B packet covers rows {g, g+1}."""
    ap = t[:].copy()
    ap.ap = bass_rust.VecI64Pair([(128, GPC), (1, 256)])
    return ap


def build_nc():
    nc = bacc.Bacc("TRN2", num_devices=NCORE, num_swdge_queues=4)

    gridT = nc.dram_tensor("gridT", [D, GPC], BF16, kind="ExternalInput")
    w_pack = nc.dram_tensor("w_pack", [128, 60], BF16, kind="ExternalInput")
    idx_in = nc.dram_tensor("idx", [NBLK, 128, CAPP // 16], I16,
                            kind="ExternalInput")
    segrel_in = nc.dram_tensor("segrel", [128, NBLK, 2 * CPB], BF16,
                               kind="ExternalInput")
    iota_in = nc.dram_tensor("iota", [128, SEGBLK], BF16, kind="ExternalInput")
    bias_in = nc.dram_tensor("bias", [128, NCH], F32, kind="ExternalInput")
    cinv_in = nc.dram_tensor("cinv", [4, 128], F32, kind="ExternalInput")
    ncnt_in = nc.dram_tensor("ncnt", [1, NBLK], I32, kind="ExternalInput")
    out_t = nc.dram_tensor("out", [SEG_PER_CORE, NCH], F32, kind="ExternalOutput")

    table = nc.dram_tensor("table", [NROWS_PAD, 128], BF16)
    partial_d = nc.dram_tensor("partial", [NBLK, 128, NCH], F16)
    rs_out = nc.dram_tensor("rs_out", [8, 64, NCH], F16)

    with tile.TileContext(nc) as tc:
        with (
            tc.tile_pool(name="const", bufs=1) as cpool,
            tc.tile_pool(name="grid", bufs=3) as gpool,
            tc.tile_pool(name="ptab", bufs=2, space="PSUM") as pt_pool,
            tc.tile_pool(name="tab", bufs=3) as tpool,
            tc.tile_pool(name="idxp", bufs=1) as ipool,
            tc.tile_pool(name="dst", bufs=6) as dpool,
            tc.tile_pool(name="oh", bufs=6) as opool,
            tc.tile_pool(name="pblk", bufs=4, space="PSUM") as pb_pool,
            tc.tile_pool(name="acc", bufs=1) as apool,
            tc.tile_pool(name="fin", bufs=2) as fpool,
        ):
            # constants
            w_sb = cpool.tile([128, 60], BF16)
            nc.sync.dma_start(w_sb[:], w_pack[:])
            segrel_sb = cpool.tile([128, NBLK, 2 * CPB], BF16)
            nc.sync.dma_start(segrel_sb[:], segrel_in[:])
            iota_sb = cpool.tile([128, SEGBLK], BF16)
            nc.sync.dma_start(iota_sb[:], iota_in[:])
            bias_sb = cpool.tile([128, NCH], F32)
            nc.sync.dma_start(bias_sb[:], bias_in[:])
            cinv_sb = cpool.tile([128, 4], F32)
            nc.sync.dma_start(cinv_sb[:], cinv_in[:].rearrange("t p -> p t"))
            ncnt_sb = cpool.tile([1, NBLK], I32)
            nc.sync.dma_start(ncnt_sb[:], ncnt_in[:])
            idx_all = ipool.tile([128, NBLK, CAPP // 16], I16)
            nc.sync.dma_start(idx_all[:], idx_in[:].rearrange("b p c -> p b c"))

            # dst pool bufs hold stale data where trimmed packets skip the
            # write; zero them once so 0*garbage in PE stays finite. Same for
            # tab bufs whose cols 19:127 ride to DRAM unwritten.
            for _ in range(6):
                dst = dpool.tile([128, CPB, 256], BF16, tag="dst")
                nc.vector.memset(dst[:], 0.0)
            for _ in range(3):
                tabp = tpool.tile([128, MTILE // 128, 128], BF16, tag="tab")
                nc.vector.memset(tabp[:], 0.0)

            # ---- Phase A: projection -> table rows [g, 0:19] ----
            ksizes = [(0, 128), (128, 128), (256, 16)]
            for mt in range(GPC // MTILE):
                gt = gpool.tile([128, 3, MTILE], BF16, tag="gt")
                for t, (k0, kn) in enumerate(ksizes):
                    nc.sync.dma_start(
                        gt[:kn, t, :],
                        gridT[k0:k0 + kn, mt * MTILE:(mt + 1) * MTILE])
                tab = tpool.tile([128, MTILE // 128, 128], BF16, tag="tab")
                for ms in range(MTILE // 128):
                    psum = pt_pool.tile([128, NCH], F32, tag="ptab")
                    for t, (k0, kn) in enumerate(ksizes):
                        nc.tensor.matmul(
                            out=psum[:],
                            lhsT=gt[:kn, t, ms * 128:(ms + 1) * 128],
                            rhs=w_sb[:kn, t * 20:t * 20 + NCH],
                            start=(t == 0), stop=(t == 2))
                    nc.scalar.copy(tab[:, ms, :NCH], psum[:])
                nc.scalar.dma_start(
                    table[mt * MTILE:(mt + 1) * MTILE]
                        .rearrange("(ms p) e -> p ms e", p=128),
                    tab[:])
            ztab = tpool.tile([128, 1, 128], BF16, tag="tab")
            nc.vector.memset(ztab[:], 0.0)
            nc.scalar.dma_start(
                table[GPC:NROWS_PAD].rearrange("(ms p) e -> p ms e", p=128),
                ztab[:])

            # ---- Phase B: paired gather + one-hot reduce ----
            part_sb = apool.tile([128, NBLK * NCH], F16)
            tbl_ap = _pair_ap(table)
            for b in range(NBLK):
                if TRIM:
                    nreg = nc.gpsimd.value_load(
                        ncnt_sb[:, b:b + 1], min_val=1, max_val=CAPP)
                else:
                    nreg = CAPP
                dst = dpool.tile([128, CPB, 256], BF16, tag="dst")
                nc.gpsimd.dma_gather(
                    dst[:], tbl_ap, idx_all[:, b, :],
                    CAPP, nreg, 256, elem_step=128, single_packet=False,
                    queue_num=b % 4)
                oh = opool.tile([128, 2 * CPB, SEGBLK], BF16, tag="oh")
                nc.vector.tensor_tensor(
                    out=oh[:],
                    in0=segrel_sb[:, b, :]
                        .rearrange("p (x o) -> p x o", o=1).broadcast_to(
                            [128, 2 * CPB, SEGBLK]),
                    in1=iota_sb[:].rearrange("p (o j) -> p o j", o=1).broadcast_to(
                            [128, 2 * CPB, SEGBLK]),
                    op=mybir.AluOpType.is_equal)
                psum_b = pb_pool.tile([128, NCH], F32, tag="pblk")
                for x in range(2 * CPB):
                    c, h = divmod(x, 2)
                    nc.tensor.matmul(
                        out=psum_b[:],
                        lhsT=oh[:, x, :],
                        rhs=dst[:, c, 128 * h:128 * h + NCH],
                        start=(x == 0), stop=(x == 2 * CPB - 1))
                nc.scalar.copy(part_sb[:, b * NCH:(b + 1) * NCH], psum_b[:])

                if b % 4 == 3:
                    h = b // 4
                    nc.sync.dma_start(
                        partial_d[h * 4:(h + 1) * 4].rearrange("b p c -> p b c"),
                        part_sb[:, h * 4 * NCH:(h + 1) * 4 * NCH]
                            .rearrange("p (b c) -> p b c", b=4))
                if RS_INLOOP and b >= 5 and (b - 5) % 4 == 0:
                    h = (b - 5) // 4
                    nc.gpsimd.collective_compute(
                        "ReduceScatter",
                        mybir.AluOpType.add,
                        replica_groups=[list(range(NCORE))],
                        ins=[partial_d[h * 4:(h + 1) * 4]],
                        outs=[rs_out[h]],
                    )
            for h in (range(7, 8) if RS_INLOOP else range(8)):
                nc.gpsimd.collective_compute(
                    "ReduceScatter",
                    mybir.AluOpType.add,
                    replica_groups=[list(range(NCORE))],
                    ins=[partial_d[h * 4:(h + 1) * 4]],
                    outs=[rs_out[h]],
                )

            # ---- Phase C: finalize owned segments ----
            rs_flat = rs_out[:].rearrange("h p c -> (h p) c")
            for t in range(4):
                fin16 = fpool.tile([128, NCH], F16, tag="fin16")
                nc.sync.dma_start(fin16[:], rs_flat[t * 128:(t + 1) * 128])
                fin = fpool.tile([128, NCH], F32, tag="fin")
                nc.vector.tensor_copy(fin[:], fin16[:])
                sc = fpool.tile([128, NCH], F32, tag="sc")
                nc.vector.tensor_scalar_mul(sc[:], fin[:], cinv_sb[:, t:t + 1])
                sc2 = fpool.tile([128, NCH], F32, tag="sc2")
                nc.vector.tensor_add(sc2[:], sc[:], bias_sb[:])
                og = fpool.tile([128, NCH], F32, tag="og")
                nc.scalar.activation(og[:], sc2[:],
                                     mybir.ActivationFunctionType.Sigmoid)
                nc.sync.dma_start(out_t[t * 128:(t + 1) * 128, :], og[:])

    nc.compile()
    return nc


def _pack_block(g, s):
    """Greedy-pair sorted grid rows g (with segment-in-block s) into 512B
    packets covering rows {p, p+1}. Returns (pidx, srelA, srelB)."""
    n = len(g)
    pidx = np.empty(n, np.int64)
    sA = np.empty(n, np.float32)
    sB = np.empty(n, np.float32)
    npk = 0
    i = 0
    while i < n:
        if i + 1 < n and g[i + 1] - g[i] == 1:
            pidx[npk], sA[npk], sB[npk] = g[i], s[i], s[i + 1]
            i += 2
        else:
            pidx[npk], sA[npk], sB[npk] = g[i], s[i], -1.0
            i += 1
        npk += 1
    return pidx[:npk], sA[:npk], sB[:npk]


def prep_inputs(class_capsules, W, b, point_idx, segment_ids, num_segments=NSEG):
    """Host-side sharding: returns in_maps (list of 8 dicts)."""
    assert int(num_segments) == NSEG
    grid = np.ascontiguousarray(class_capsules.reshape(GRID, D), np.float32)
    point_idx = np.asarray(point_idx, np.int64)
    segment_ids = np.asarray(segment_ids, np.int64)
    W = np.asarray(W, np.float32)
    b = np.asarray(b, np.float32)

    w_pack = np.zeros((128, 60), ml_dtypes.bfloat16)
    w20 = np.concatenate([W, np.zeros((D, 1), np.float32)], 1)  # [272, 20]
    w_pack[:, 0:20] = w20[0:128].astype(ml_dtypes.bfloat16)
    w_pack[:, 20:40] = w20[128:256].astype(ml_dtypes.bfloat16)
    w_pack[0:16, 40:60] = w20[256:272].astype(ml_dtypes.bfloat16)

    iota = np.tile(np.arange(SEGBLK, dtype=np.float32), (128, 1)).astype(
        ml_dtypes.bfloat16)
    bias_rep = np.tile(b[None, :], (128, 1)).astype(np.float32)

    counts = np.bincount(segment_ids, minlength=NSEG).astype(np.float32)
    cinv_all = 1.0 / np.maximum(counts, 1.0)

    in_maps = []
    for k in range(NCORE):
        sel = (point_idx >= k * GPC) & (point_idx < (k + 1) * GPC)
        lidx = point_idx[sel] - k * GPC
        lseg = segment_ids[sel]          # sorted ascending
        blk = lseg >> 7
        srel = (lseg & 127).astype(np.float32)

        idx_pad = np.full((NBLK, CAPP), -1 if TRIM else 0, np.int16)
        srel_pad = np.full((NBLK, CAPP, 2), -1.0, np.float32)
        npk_arr = np.zeros(NBLK, np.int32)
        for bb in range(NBLK):
            m = blk == bb
            g = lidx[m]
            s = srel[m]
            order = np.argsort(g, kind="stable")
            pidx, sA, sB = _pack_block(g[order], s[order])
            npk = len(pidx)
            assert npk <= CAPP, f"core {k} block {bb}: {npk} > CAPP"
            npk_arr[bb] = npk
            idx_pad[bb, :npk] = pidx.astype(np.int16)
            srel_pad[bb, :npk, 0] = sA
            srel_pad[bb, :npk, 1] = sB

        idxw = np.empty((NBLK, 128, CAPP // 16), np.int16)
        for bb in range(NBLK):
            idxw[bb] = np.tile(idx_pad[bb].reshape(-1, 16).T, (8, 1))

        # segrel [128, NBLK, 2*CPB]: value at [p, b, 2*c+h] = srel of slot
        # (b, chunk c, packet p, half h)
        segrel_arr = (srel_pad.reshape(NBLK, CPB, 128, 2)
                      .transpose(2, 0, 1, 3).reshape(128, NBLK, 2 * CPB)
                      .astype(ml_dtypes.bfloat16))

        # cinv for this core's finalize rows: row r of rs_flat [512] ->
        # seg = 512*(r//64) + 64*k + (r%64)
        r = np.arange(512)
        cinv_core = cinv_all[512 * (r // 64) + 64 * k + (r % 64)]

        gridT_k = np.ascontiguousarray(
            grid[k * GPC:(k + 1) * GPC].T).astype(ml_dtypes.bfloat16)

        in_maps.append({
            "gridT": gridT_k,
            "w_pack": w_pack,
            "idx": idxw,
            "segrel": np.ascontiguousarray(segrel_arr),
            "iota": iota,
            "bias": bias_rep,
            "cinv": cinv_core.reshape(4, 128).astype(np.float32),
            "ncnt": npk_arr.reshape(1, NBLK),
        })
    return in_maps


def assemble(results):
    out = np.empty((NSEG, NCH), np.float32)
    for k in range(NCORE):
        r = results[k]["out"]  # [512, 19]: row i -> chunk h=i//64, j=i%64
        for h in range(8):
            out[4 * h * 128 + k * 64: 4 * h * 128 + (k + 1) * 64] = \
                r[h * 64:(h + 1) * 64]
    return out


_NC_CACHE = {}


def kernel(class_capsules, W, b, point_idx, segment_ids, num_segments):
    """Full-input entry point: shard across 8 NeuronCores, run, reassemble."""
    from concourse.bass_utils import run_bass_kernel_spmd

    in_maps = prep_inputs(np.asarray(class_capsules), np.asarray(W),
                          np.asarray(b), np.asarray(point_idx),
                          np.asarray(segment_ids), int(num_segments))
    if "nc" not in _NC_CACHE:
        _NC_CACHE["nc"] = build_nc()
    res = run_bass_kernel_spmd(_NC_CACHE["nc"], in_maps, list(range(NCORE)))
    return assemble(res.results)


# revision 8
# speedup vs baseline: 1.2300x; 1.1346x over previous
"""CapsuleModel2 segment-reduce kernel for 8 TRN2 NeuronCores (v3).

Math (per reference):
    feats  = class_capsules.reshape(65536, 272)[point_idx]        # [P, 272]
    sums   = segment_sum(feats, segment_ids, 4096)                # [4096, 272]
    counts = segment_sum(ones)                                    # [4096]
    out    = sigmoid((sums / max(counts,1)) @ W + b)              # [4096, 19]

Identity used: (sums @ W) = segment_sum(feats @ W): project the grid to 19
channels on the PE first, then gather narrow rows.

Distribution (table-sharded):
  - core k owns grid rows [k*8192, (k+1)*8192) and the points hitting them
  - per 128-segment block, the core's points are gathered and reduced into
    psum[128seg, 19] via one-hot matmuls; partials for all 4096 segments are
    ReduceScattered (partition-major split) so each core finalizes 512 segs
  - counts come from a host bincount (index arithmetic only), shipped as
    reciprocals, so no count column is needed on device

Gather economics (one DMA queue packet costs ~the same for 256B and 512B):
  - each gather index fetches TWO adjacent 256B table rows (elem_step=128,
    elem_size=256 -> one 512B packet covering rows {g, g+1}); points sorted
    by grid row within a block are greedily paired so one packet often
    serves two points; unused slots carry srel=-1 and die in the one-hot
  - gather sizes are static per block (max packet count over cores, x16),
    so pad packets beyond that bound cost no DMA traffic
"""

import os
import sys

for _p in ('/opt/trn_rl_repo',):
    if _p not in sys.path:
        sys.path.insert(0, _p)

import numpy as np
import ml_dtypes

import bass_rust
import concourse.bacc as bacc
import concourse.bass as bass
import concourse.mybir as mybir
import concourse.tile as tile

BF16 = mybir.dt.bfloat16
F32 = mybir.dt.float32
F16 = mybir.dt.float16
I16 = mybir.dt.int16

NCORE = 8
GRID = 65536
GPC = GRID // NCORE          # 8192 grid rows per core
D = 272                      # capsule feature dim
NCH = 19                     # output channels
NSEG = 4096
SEGBLK = 128                 # segments per block
NBLK = NSEG // SEGBLK        # 32
CAPP = 1920                  # tile-size bound on packets per (core, block)
CPB = CAPP // 128            # 15 packet-chunks per block
NROWS_PAD = GPC + 128        # table rows incl. zero pad row for pair overread
MTILE = 1024                 # grid columns per projection step
RS_INLOOP = os.environ.get("KRSIL", "1") == "1"


def _pair_ap(t):
    """Overlapping AP over table [NROWS_PAD,128]: row stride 128 elems,
    row view 256 elems -> one 512B packet covers rows {g, g+1}."""
    ap = t[:].copy()
    ap.ap = bass_rust.VecI64Pair([(128, GPC), (1, 256)])
    return ap


def build_nc(nb16):
    """nb16: per-block static gather sizes (multiples of 16, <= CAPP)."""
    nc = bacc.Bacc("TRN2", num_devices=NCORE, num_swdge_queues=4)

    gridT = nc.dram_tensor("gridT", [D, GPC], BF16, kind="ExternalInput")
    w_pack = nc.dram_tensor("w_pack", [128, 60], BF16, kind="ExternalInput")
    idx_in = nc.dram_tensor("idx", [128, NBLK, CAPP // 16], I16,
                            kind="ExternalInput")
    segrel_in = nc.dram_tensor("segrel", [128, NBLK, 2 * CPB], BF16,
                               kind="ExternalInput")
    iota_in = nc.dram_tensor("iota", [128, SEGBLK], BF16, kind="ExternalInput")
    bias_in = nc.dram_tensor("bias", [128, NCH], F32, kind="ExternalInput")
    cinv_in = nc.dram_tensor("cinv", [128, 4], F32, kind="ExternalInput")
    out_t = nc.dram_tensor("out", [128, 4, NCH], F32, kind="ExternalOutput")

    table = nc.dram_tensor("table", [NROWS_PAD, 128], BF16)
    partial_d = nc.dram_tensor("partial", [8, 128, 4, NCH], F16)
    rs_out = nc.dram_tensor("rs_out", [8, 16, 4, NCH], F16)

    with tile.TileContext(nc) as tc:
        with (
            tc.tile_pool(name="const", bufs=1) as cpool,
            tc.tile_pool(name="grid", bufs=3) as gpool,
            tc.tile_pool(name="ptab", bufs=2, space="PSUM") as pt_pool,
            tc.tile_pool(name="tab", bufs=3) as tpool,
            tc.tile_pool(name="dst", bufs=6) as dpool,
            tc.tile_pool(name="oh", bufs=6) as opool,
            tc.tile_pool(name="pblk", bufs=4, space="PSUM") as pb_pool,
            tc.tile_pool(name="acc", bufs=1) as apool,
            tc.tile_pool(name="fin", bufs=1) as fpool,
        ):
            # constants (idx is pre-transposed on host: one contiguous load)
            w_sb = cpool.tile([128, 60], BF16)
            nc.sync.dma_start(w_sb[:], w_pack[:])
            segrel_sb = cpool.tile([128, NBLK, 2 * CPB], BF16)
            nc.sync.dma_start(segrel_sb[:], segrel_in[:])
            iota_sb = cpool.tile([128, SEGBLK], BF16)
            nc.sync.dma_start(iota_sb[:], iota_in[:])
            bias_sb = cpool.tile([128, NCH], F32)
            nc.sync.dma_start(bias_sb[:], bias_in[:])
            cinv_sb = cpool.tile([128, 4], F32)
            nc.sync.dma_start(cinv_sb[:], cinv_in[:])
            idx_all = cpool.tile([128, NBLK, CAPP // 16], I16)
            nc.sync.dma_start(idx_all[:], idx_in[:])

            # one-off: make dst pool contents finite (slots beyond a block's
            # gather size are stale and ride into the PE where srel=-1 zeroes
            # them; 0*garbage must stay finite)
            for _ in range(6):
                dst = dpool.tile([128, CPB, 256], BF16, tag="dst")
                nc.vector.memset(dst[:], 0.0)

            # ---- Phase A: projection -> table rows [g, 0:19] ----
            # gridT loads split across the two HWDGE queues (sync + scalar)
            ld_engs = [nc.sync, nc.scalar, nc.sync]
            ksizes = [(0, 128), (128, 128), (256, 16)]
            for mt in range(GPC // MTILE):
                gt = gpool.tile([128, 3, MTILE], BF16, tag="gt")
                for t, (k0, kn) in enumerate(ksizes):
                    ld_engs[t].dma_start(
                        gt[:kn, t, :],
                        gridT[k0:k0 + kn, mt * MTILE:(mt + 1) * MTILE])
                tab = tpool.tile([128, MTILE // 128, 128], BF16, tag="tab")
                nc.vector.memset(tab[:, :, NCH:], 0.0)
                for ms in range(MTILE // 128):
                    psum = pt_pool.tile([128, NCH], F32, tag="ptab")
                    for t, (k0, kn) in enumerate(ksizes):
                        nc.tensor.matmul(
                            out=psum[:],
                            lhsT=gt[:kn, t, ms * 128:(ms + 1) * 128],
                            rhs=w_sb[:kn, t * 20:t * 20 + NCH],
                            start=(t == 0), stop=(t == 2))
                    nc.vector.tensor_copy(tab[:, ms, :NCH], psum[:])
                nc.scalar.dma_start(
                    table[mt * MTILE:(mt + 1) * MTILE]
                        .rearrange("(ms p) e -> p ms e", p=128),
                    tab[:])
            ztab = tpool.tile([128, MTILE // 128, 128], BF16, tag="tab")
            nc.vector.memset(ztab[:], 0.0)
            nc.scalar.dma_start(
                table[GPC:NROWS_PAD].rearrange("(ms p) e -> p ms e", p=128),
                ztab[:, :1, :])

            # ---- Phase B: paired gather + one-hot reduce ----
            part_sb = apool.tile([128, NBLK * NCH], F16)
            tbl_ap = _pair_ap(table)
            for b in range(NBLK):
                nb = int(nb16[b])
                cb = (nb + 127) // 128
                dst = dpool.tile([128, CPB, 256], BF16, tag="dst")
                nc.gpsimd.dma_gather(
                    dst[:, :cb, :], tbl_ap, idx_all[:, b, :nb // 16],
                    nb, nb, 256, elem_step=128, single_packet=False,
                    queue_num=b % 4)
                oh = opool.tile([128, 2 * CPB, SEGBLK], BF16, tag="oh")
                nc.vector.tensor_tensor(
                    out=oh[:, :2 * cb, :],
                    in0=segrel_sb[:, b, :2 * cb]
                        .rearrange("p (x o) -> p x o", o=1).broadcast_to(
                            [128, 2 * cb, SEGBLK]),
                    in1=iota_sb[:].rearrange("p (o j) -> p o j", o=1).broadcast_to(
                            [128, 2 * cb, SEGBLK]),
                    op=mybir.AluOpType.is_equal)
                psum_b = pb_pool.tile([128, NCH], F32, tag="pblk")
                for x in range(2 * cb):
                    c, h = divmod(x, 2)
                    nc.tensor.matmul(
                        out=psum_b[:],
                        lhsT=oh[:, x, :],
                        rhs=dst[:, c, 128 * h:128 * h + NCH],
                        start=(x == 0), stop=(x == 2 * cb - 1))
                nc.scalar.copy(part_sb[:, b * NCH:(b + 1) * NCH], psum_b[:])

                if b % 4 == 3:
                    h = b // 4
                    nc.sync.dma_start(
                        partial_d[h].rearrange("p b c -> p (b c)"),
                        part_sb[:, h * 4 * NCH:(h + 1) * 4 * NCH])
                if RS_INLOOP and b >= 5 and (b - 5) % 4 == 0:
                    h = (b - 5) // 4
                    nc.gpsimd.collective_compute(
                        "ReduceScatter",
                        mybir.AluOpType.add,
                        replica_groups=[list(range(NCORE))],
                        ins=[partial_d[h]],
                        outs=[rs_out[h]],
                    )
            for h in (range(7, 8) if RS_INLOOP else range(8)):
                nc.gpsimd.collective_compute(
                    "ReduceScatter",
                    mybir.AluOpType.add,
                    replica_groups=[list(range(NCORE))],
                    ins=[partial_d[h]],
                    outs=[rs_out[h]],
                )

            # ---- Phase C: finalize owned segments (one load, one store) ----
            fin16 = fpool.tile([128, 4, NCH], F16)
            nc.sync.dma_start(
                fin16[:].rearrange("p b c -> p (b c)"),
                rs_out[:].rearrange("h p b c -> (h p) (b c)"))
            fin = fpool.tile([128, 4, NCH], F32)
            nc.vector.tensor_copy(fin[:], fin16[:])
            sc = fpool.tile([128, 4, NCH], F32)
            nc.vector.tensor_tensor(
                out=sc[:],
                in0=fin[:],
                in1=cinv_sb[:].rearrange("p (b o) -> p b o", o=1)
                    .broadcast_to([128, 4, NCH]),
                op=mybir.AluOpType.mult)
            sc2 = fpool.tile([128, 4, NCH], F32)
            nc.vector.tensor_tensor(
                out=sc2[:],
                in0=sc[:],
                in1=bias_sb[:].rearrange("p (o c) -> p o c", o=1)
                    .broadcast_to([128, 4, NCH]),
                op=mybir.AluOpType.add)
            og = fpool.tile([128, 4, NCH], F32)
            nc.scalar.activation(og[:], sc2[:],
                                 mybir.ActivationFunctionType.Sigmoid)
            nc.sync.dma_start(out_t[:], og[:])

    nc.compile()
    return nc


def _pack_block(g, s):
    """Greedy-pair sorted grid rows g (with segment-in-block s) into 512B
    packets covering rows {p, p+1}. Returns (pidx, srelA, srelB)."""
    n = len(g)
    pidx = np.empty(n, np.int64)
    sA = np.empty(n, np.float32)
    sB = np.empty(n, np.float32)
    npk = 0
    i = 0
    while i < n:
        if i + 1 < n and g[i + 1] - g[i] == 1:
            pidx[npk], sA[npk], sB[npk] = g[i], s[i], s[i + 1]
            i += 2
        else:
            pidx[npk], sA[npk], sB[npk] = g[i], s[i], -1.0
            i += 1
        npk += 1
    return pidx[:npk], sA[:npk], sB[:npk]


def prep_inputs(class_capsules, W, b, point_idx, segment_ids, num_segments=NSEG):
    """Host-side sharding: returns (in_maps, nb16)."""
    assert int(num_segments) == NSEG
    grid = np.ascontiguousarray(class_capsules.reshape(GRID, D), np.float32)
    point_idx = np.asarray(point_idx, np.int64)
    segment_ids = np.asarray(segment_ids, np.int64)
    W = np.asarray(W, np.float32)
    b = np.asarray(b, np.float32)

    w_pack = np.zeros((128, 60), ml_dtypes.bfloat16)
    w20 = np.concatenate([W, np.zeros((D, 1), np.float32)], 1)  # [272, 20]
    w_pack[:, 0:20] = w20[0:128].astype(ml_dtypes.bfloat16)
    w_pack[:, 20:40] = w20[128:256].astype(ml_dtypes.bfloat16)
    w_pack[0:16, 40:60] = w20[256:272].astype(ml_dtypes.bfloat16)

    iota = np.tile(np.arange(SEGBLK, dtype=np.float32), (128, 1)).astype(
        ml_dtypes.bfloat16)
    bias_rep = np.tile(b[None, :], (128, 1)).astype(np.float32)

    counts = np.bincount(segment_ids, minlength=NSEG).astype(np.float32)
    cinv_all = 1.0 / np.maximum(counts, 1.0)

    packed = []
    npk_all = np.zeros((NCORE, NBLK), np.int64)
    for k in range(NCORE):
        sel = (point_idx >= k * GPC) & (point_idx < (k + 1) * GPC)
        lidx = point_idx[sel] - k * GPC
        lseg = segment_ids[sel]          # sorted ascending
        blk = lseg >> 7
        srel = (lseg & 127).astype(np.float32)
        blocks = []
        for bb in range(NBLK):
            m = blk == bb
            g = lidx[m]
            s = srel[m]
            order = np.argsort(g, kind="stable")
            pidx, sA, sB = _pack_block(g[order], s[order])
            assert len(pidx) <= CAPP, f"core {k} block {bb}: {len(pidx)}"
            npk_all[k, bb] = len(pidx)
            blocks.append((pidx, sA, sB))
        packed.append(blocks)

    nb16 = np.minimum(((npk_all.max(0) + 15) // 16) * 16, CAPP).astype(np.int64)

    in_maps = []
    for k in range(NCORE):
        idx_pad = np.zeros((NBLK, CAPP), np.int16)
        srel_pad = np.full((NBLK, CAPP, 2), -1.0, np.float32)
        for bb, (pidx, sA, sB) in enumerate(packed[k]):
            npk = len(pidx)
            idx_pad[bb, :npk] = pidx.astype(np.int16)
            srel_pad[bb, :npk, 0] = sA
            srel_pad[bb, :npk, 1] = sB

        # idx pre-transposed to the device layout [128, NBLK, CAPP/16]:
        # partition p of block b holds packets {c*128+p} wrapped by 16
        idxw = np.empty((128, NBLK, CAPP // 16), np.int16)
        for bb in range(NBLK):
            idxw[:, bb, :] = np.tile(idx_pad[bb].reshape(-1, 16).T, (8, 1))

        segrel_arr = (srel_pad.reshape(NBLK, CPB, 128, 2)
                      .transpose(2, 0, 1, 3).reshape(128, NBLK, 2 * CPB)
                      .astype(ml_dtypes.bfloat16))

        # cinv [128, 4]: partition r=(h*16+pl), col b -> seg of rs row
        r = np.arange(128)
        h, pl = r // 16, r % 16
        segmap = (128 * (4 * h[:, None] + np.arange(4)[None, :])
                  + 16 * k + pl[:, None])                      # [128, 4]
        cinv_core = cinv_all[segmap]

        gridT_k = np.ascontiguousarray(
            grid[k * GPC:(k + 1) * GPC].T).astype(ml_dtypes.bfloat16)

        in_maps.append({
            "gridT": gridT_k,
            "w_pack": w_pack,
            "idx": idxw,
            "segrel": np.ascontiguousarray(segrel_arr),
            "iota": iota,
            "bias": bias_rep,
            "cinv": cinv_core.astype(np.float32),
        })
    return in_maps, nb16


def assemble(results):
    out = np.empty((NSEG, NCH), np.float32)
    r = np.arange(128)
    h, pl = r // 16, r % 16
    for k in range(NCORE):
        o = results[k]["out"]  # [128, 4, 19]
        segmap = (128 * (4 * h[:, None] + np.arange(4)[None, :])
                  + 16 * k + pl[:, None])
        out[segmap.reshape(-1)] = o.reshape(-1, NCH)
    return out


_NC_CACHE = {}


def kernel(class_capsules, W, b, point_idx, segment_ids, num_segments):
    """Full-input entry point: shard across 8 NeuronCores, run, reassemble."""
    from concourse.bass_utils import run_bass_kernel_spmd

    in_maps, nb16 = prep_inputs(np.asarray(class_capsules), np.asarray(W),
                                np.asarray(b), np.asarray(point_idx),
                                np.asarray(segment_ids), int(num_segments))
    key = tuple(nb16.tolist())
    if _NC_CACHE.get("key") != key:
        _NC_CACHE["nc"] = build_nc(nb16)
        _NC_CACHE["key"] = key
    res = run_bass_kernel_spmd(_NC_CACHE["nc"], in_maps, list(range(NCORE)))
    return assemble(res.results)
